# revision 11
# baseline (speedup 1.0000x reference)
"""Trainium2 Bass kernel for the KGCN-style GNN message-passing problem.

Reference (B=16384, K=32 neighbors, D=64):
    u = user_emb[users]; i = entity_emb[items]
    predict = sigmoid(sum(u*i, -1))
    t_r = concat([relation_emb[neighbor_rel], entity_emb[neighbor_tail]], -1)
    false_item = sigmoid(t_r.reshape(B,-1) @ gen_W.T + gen_b)
    false_predict = sigmoid(sum(u*false_item, -1))
    out = BCE(predict, label) + BCE(false_predict, label)      # scalar

Data-parallel over batch across 8 cores (2048 rows each); tables replicated.

The dominant work is gathering 65536+2048 random entity rows per core. The
only fast bulk-gather primitive (InstDMAGatherAnt) takes int16 indices, so it
only addresses 32768-row windows with a zero base (nonzero AP bases corrupt
descriptors - HW-probed), and its output placement is list-order. Two levels:

  level 1 (31 instrs): per entity window (separate zero-base DRAM tensors),
      one gather whose list is [seg0-sublist | seg1 | seg2], each sublist
      statically sized (1024/1024/768, row-0 padded) -> SBUF buffer ->
      3 stage-out DMAs append window-blocks to 3 compact DRAM staging
      tensors (each < 32768 rows so level 2 can address them with int16).
  level 2: per batch-tile, a transpose-mode dma_gather from its staging
      segment pulls rows in destination order directly as feature-major
      matmul tiles [128 feat, 128 batch] - no on-chip transposes at all.
      Items are pulled with one non-transpose gather (row-major for dots).

Rows are stored hi||lo bf16 (hi=bf16(x), lo=bf16(x-hi), 256B rows): the
transpose-mode gather (16-bit granularity) lands hi dims on partitions 0-63
and lo dims on 64-127, and the projection matmul contracts both against
duplicated weight rows, recovering ~f32 accuracy at bf16 dtype.

relation_emb is tiny (64 rows, int16-addressable directly), so rel features
come straight from transpose-mode gathers on the hi||lo relation table.
User rows use one-offset-per-partition indirect DMAs (2048 rows only).
Each core emits per-row BCE term sums [128]; the host sums and scales -1/B.
"""

import os
import sys

for _p in ("/opt/trn_rl_repo",):
    if _p not in sys.path and os.path.isdir(_p):
        sys.path.append(_p)

import numpy as np
import ml_dtypes

import concourse.bass as bass
import concourse.mybir as mybir
import concourse.tile as tile
from concourse import bacc

B = 16384
K = 32
D = 64
E = 2 * D            # hi||lo row elems (bf16) = 256 B
NC = 8
BL = B // NC         # rows per core
T = BL // 128        # batch tiles per core (16)
N_USER = 500_000
N_ENTITY = 1_000_000
N_REL = 64
W = 32768            # dma_gather window rows
NW = (N_ENTITY + W - 1) // W     # 31 entity windows

# staging segments: btiles 0-5 / 6-11 / 12-15+items
SEG_BTILES = [range(0, 6), range(6, 12), range(12, 16)]
SEG_OF_BTILE = [0] * 6 + [1] * 6 + [2] * 4
CAPS = [1024, 1024, 768]         # per-(window, seg) list capacity (x128)
CAP_OFF = [0, 1024, 2048]
NIDX1 = sum(CAPS)                # 2816 idxs per level-1 instruction
SLOTS1 = NIDX1 // 128            # 22 slots in the window buffer

AF = mybir.ActivationFunctionType
OP = mybir.AluOpType
BF16 = ml_dtypes.bfloat16

_module_cache = {}
LAST_RESULTS = None


def _install_ntff_shim():
    import types

    if "antenv.axon_hooks" in sys.modules:
        return
    mod = types.ModuleType("antenv.axon_hooks")
    mod._hook = None
    mod.set_axon_ntff_profile_hook = lambda h: setattr(mod, "_hook", h)
    mod.get_axon_ntff_profile_hook = lambda: mod._hook
    sys.modules["antenv.axon_hooks"] = mod
    try:
        import antenv

        antenv.axon_hooks = mod
        from trn_agent_boot.trn_boot import _ntff_profile_via_ctypes

        mod.set_axon_ntff_profile_hook(
            _ntff_profile_via_ctypes("/opt/axon/libaxon_pjrt.so")
        )
    except Exception:
        pass


def _build_module():
    from contextlib import ExitStack

    no_l1 = bool(int(os.environ.get("V2_NO_L1", "0")))
    no_l2 = bool(int(os.environ.get("V2_NO_L2", "0")))
    no_rel = bool(int(os.environ.get("V2_NO_REL", "0")))
    no_items = bool(int(os.environ.get("V2_NO_ITEMS", "0")))

    nc = bacc.Bacc(
        "TRN2",
        target_bir_lowering=False,
        debug=False,
        enable_asserts=False,
        num_devices=NC,
    )
    f32, bf16 = mybir.dt.float32, mybir.dt.bfloat16
    i16, i32 = mybir.dt.int16, mybir.dt.int32

    entw = [
        nc.dram_tensor(f"entw{w}", [W, E], bf16, kind="ExternalInput").ap()
        for w in range(NW)
    ]
    usr2 = nc.dram_tensor("usr2", [N_USER, E], bf16, kind="ExternalInput").ap()
    rel2 = nc.dram_tensor("rel2", [N_REL, E], bf16, kind="ExternalInput").ap()
    l1idx = nc.dram_tensor(
        "l1idx", [128, NW * (NIDX1 // 16)], i16, kind="ExternalInput"
    ).ap()
    l2idx = nc.dram_tensor("l2idx", [128, T * 256], i16, kind="ExternalInput").ap()
    relidx = nc.dram_tensor("relidx", [128, T * 256], i16, kind="ExternalInput").ap()
    itidx = nc.dram_tensor("itidx", [128, BL // 16], i16, kind="ExternalInput").ap()
    uidx = nc.dram_tensor("uidx", [128, T], i32, kind="ExternalInput").ap()
    yv = nc.dram_tensor("yv", [128, T], f32, kind="ExternalInput").ap()
    wtail = nc.dram_tensor("wtail", [128, K * D], bf16, kind="ExternalInput").ap()
    wrel = nc.dram_tensor("wrel", [128, K * D], bf16, kind="ExternalInput").ap()
    biasw = nc.dram_tensor("biasw", [1, D], bf16, kind="ExternalInput").ap()
    loss = nc.dram_tensor("loss", [128, 1], f32, kind="ExternalOutput").ap()

    with tile.TileContext(nc) as tc, ExitStack() as ctx:
        const = ctx.enter_context(tc.tile_pool(name="const", bufs=1))
        winp = ctx.enter_context(tc.tile_pool(name="winp", bufs=4))
        lhsp = ctx.enter_context(tc.tile_pool(name="lhsp", bufs=3))
        apsum = ctx.enter_context(tc.tile_pool(name="apsum", bufs=2, space="PSUM"))
        small = ctx.enter_context(tc.tile_pool(name="small", bufs=2))
        dram = ctx.enter_context(tc.tile_pool(name="dram", bufs=1, space="DRAM"))

        # ---- persistent loads ----
        l1_sb = const.tile([128, NW * (NIDX1 // 16)], i16)
        nc.sync.dma_start(l1_sb[:], l1idx)
        l2_sb = const.tile([128, T * 256], i16)
        nc.sync.dma_start(l2_sb[:], l2idx)
        rix_sb = const.tile([128, T * 256], i16)
        nc.sync.dma_start(rix_sb[:], relidx)
        itix_sb = const.tile([128, BL // 16], i16)
        nc.sync.dma_start(itix_sb[:], itidx)
        uix_sb = const.tile([128, T], i32)
        nc.sync.dma_start(uix_sb[:], uidx)
        y_sb = const.tile([128, T], f32)
        nc.sync.dma_start(y_sb[:], yv)
        wt_sb = const.tile([128, K * D], bf16)
        nc.sync.dma_start(wt_sb[:], wtail)
        wr_sb = const.tile([128, K * D], bf16)
        nc.sync.dma_start(wr_sb[:], wrel)
        bias_sb = const.tile([1, D], bf16)
        nc.sync.dma_start(bias_sb[:], biasw)
        ones1 = const.tile([1, 128], bf16)
        nc.gpsimd.memset(ones1[:], 1.0)

        # ---- users via per-partition indirect gathers ----
        u_g = const.tile([128, T, E], bf16)
        for t in range(T):
            nc.gpsimd.indirect_dma_start(
                out=u_g[:, t, :],
                out_offset=None,
                in_=usr2,
                in_offset=bass.IndirectOffsetOnAxis(ap=uix_sb[:, t : t + 1], axis=0),
            )
        u2 = const.tile([128, T * D], f32)
        nc.vector.tensor_tensor(
            u2[:].rearrange("p (t d) -> p t d", t=T),
            u_g[:, :, 0:D],
            u_g[:, :, D:E],
            op=OP.add,
        )

        # ---- staging DRAM segments ----
        stg = [
            dram.tile([NW * CAPS[s], E], bf16, tag=f"stg{s}", name=f"stg{s}")
            for s in range(3)
        ]

        # ---- level 1: 31 windowed gathers -> compact staging ----
        for w in range(NW if not no_l1 else 0):
            buf = winp.tile([128, SLOTS1, E], bf16, tag="winbuf")
            nc.gpsimd.dma_gather(
                out_ap=buf[:],
                in_ap=entw[w][:],
                idxs_ap=l1_sb[:, w * (NIDX1 // 16) : (w + 1) * (NIDX1 // 16)],
                num_idxs=NIDX1,
                num_idxs_reg=NIDX1,
                elem_size=E,
                single_packet=False,
            )
            for s in range(3):
                cslots = CAPS[s] // 128
                c0 = CAP_OFF[s] // 128
                nc.sync.dma_start(
                    stg[s][w * CAPS[s] : (w + 1) * CAPS[s], :].rearrange(
                        "(p q) e -> p q e", p=128
                    ),
                    buf[:, c0 : c0 + cslots, :],
                )

        # ---- items: one non-transpose level-2 gather from stg2 ----
        i_g = const.tile([128, T, E], bf16)
        if no_items:
            nc.gpsimd.memset(i_g[:], 0.0)
        else:
            nc.gpsimd.dma_gather(
                out_ap=i_g[:],
                in_ap=stg[2][:],
                idxs_ap=itix_sb[:],
                num_idxs=BL,
                num_idxs_reg=BL,
                elem_size=E,
                single_packet=False,
            )
        i2 = const.tile([128, T * D], f32)
        nc.vector.tensor_tensor(
            i2[:].rearrange("p (t d) -> p t d", t=T),
            i_g[:, :, 0:D],
            i_g[:, :, D:E],
            op=OP.add,
        )

        zp = const.tile([128, T], f32)
        zf = const.tile([128, T], f32)

        # ---- per btile: transposed gathers + projection matmuls ----
        for t in range(T):
            s = SEG_OF_BTILE[t]
            tailT = lhsp.tile([128, K * 128], bf16, tag="tailT")
            if no_l2:
                nc.gpsimd.memset(tailT[:], 0.0)
            else:
                nc.gpsimd.dma_gather(
                    out_ap=tailT[:].rearrange("p (a n) -> p a n", a=1),
                    in_ap=stg[s][:],
                    idxs_ap=l2_sb[:, t * 256 : (t + 1) * 256],
                    num_idxs=K * 128,
                    num_idxs_reg=K * 128,
                    elem_size=E,
                    transpose=True,
                    single_packet=False,
                )
            relT = lhsp.tile([128, K * 128], bf16, tag="relT")
            if no_rel:
                nc.gpsimd.memset(relT[:], 0.0)
            else:
                nc.gpsimd.dma_gather(
                    out_ap=relT[:].rearrange("p (a n) -> p a n", a=1),
                    in_ap=rel2[:],
                    idxs_ap=rix_sb[:, t * 256 : (t + 1) * 256],
                    num_idxs=K * 128,
                    num_idxs_reg=K * 128,
                    elem_size=E,
                    transpose=True,
                    single_packet=False,
                )

            acc = apsum.tile([128, D], f32, tag="acc")
            for k in range(K):
                nc.tensor.matmul(
                    acc[:],
                    lhsT=tailT[:, k * 128 : (k + 1) * 128],
                    rhs=wt_sb[:, k * D : (k + 1) * D],
                    start=(k == 0),
                    stop=False,
                )
                nc.tensor.matmul(
                    acc[:],
                    lhsT=relT[:, k * 128 : (k + 1) * 128],
                    rhs=wr_sb[:, k * D : (k + 1) * D],
                    start=False,
                    stop=False,
                )
            nc.tensor.matmul(
                acc[:], lhsT=ones1[:], rhs=bias_sb[:], start=False, stop=True
            )

            fi = small.tile([128, D], f32, tag="fi")
            nc.scalar.activation(fi[:], acc[:], AF.Sigmoid)
            prod = small.tile([128, D], f32, tag="prod")
            nc.vector.tensor_tensor(
                prod[:],
                u2[:].rearrange("p (t d) -> p t d", t=T)[:, t, :],
                fi[:],
                op=OP.mult,
            )
            nc.vector.tensor_reduce(
                zf[:, t : t + 1], prod[:], axis=mybir.AxisListType.X, op=OP.add
            )
            prod2 = small.tile([128, D], f32, tag="prod2")
            nc.vector.tensor_tensor(
                prod2[:],
                u2[:].rearrange("p (t d) -> p t d", t=T)[:, t, :],
                i2[:].rearrange("p (t d) -> p t d", t=T)[:, t, :],
                op=OP.mult,
            )
            nc.vector.tensor_reduce(
                zp[:, t : t + 1], prod2[:], axis=mybir.AxisListType.X, op=OP.add
            )

        # ---- BCE tail (logits tiny: |z| < 0.5 -> the reference's
        #      clip(log, -100) can never fire; skip the clamp) ----
        def bce_terms(z, name):
            p_ = const.tile([128, T], f32, tag=f"p_{name}")
            nc.scalar.activation(p_[:], z[:], AF.Sigmoid)
            lp = const.tile([128, T], f32, tag=f"lp_{name}")
            nc.scalar.activation(lp[:], p_[:], AF.Ln)
            lm = const.tile([128, T], f32, tag=f"lm_{name}")
            nc.scalar.activation(lm[:], p_[:], AF.Ln, bias=1.0, scale=-1.0)
            d_ = const.tile([128, T], f32, tag=f"d_{name}")
            nc.vector.tensor_tensor(d_[:], lp[:], lm[:], op=OP.subtract)
            t_ = const.tile([128, T], f32, tag=f"t_{name}")
            nc.vector.tensor_tensor(t_[:], y_sb[:], d_[:], op=OP.mult)
            t2 = const.tile([128, T], f32, tag=f"t2_{name}")
            nc.vector.tensor_tensor(t2[:], t_[:], lm[:], op=OP.add)
            return t2

        tp_ = bce_terms(zp, "zp")
        tf_ = bce_terms(zf, "zf")
        tot = const.tile([128, T], f32)
        nc.vector.tensor_add(tot[:], tp_[:], tf_[:])
        lsum = const.tile([128, 1], f32)
        nc.vector.tensor_reduce(lsum[:], tot[:], axis=mybir.AxisListType.X, op=OP.add)
        nc.sync.dma_start(loss, lsum[:])

    nc.compile()
    return nc


def _get_module():
    if "nc" not in _module_cache:
        _module_cache["nc"] = _build_module()
    return _module_cache["nc"]


def _hilo(x):
    """f32 [N, D] -> hi||lo bf16 [N, 2D]."""
    x = np.asarray(x, dtype=np.float32)
    hi = x.astype(BF16)
    lo = (x - hi.astype(np.float32)).astype(BF16)
    return np.ascontiguousarray(np.concatenate([hi, lo], axis=1))


def _pack_idx16(lst):
    """int16 list (len mult of 16) -> [128, len/16] wrapped+replicated tile."""
    n = len(lst)
    a = np.asarray(lst, dtype=np.int16).reshape(n // 16, 16).T  # [16, n/16]
    return np.tile(a, (8, 1))


def _core_indices(nt, nr, items_c):
    """Build level-1 lists, level-2/rel/items idx for one core.

    nt/nr: [T, 128, K] int64; items_c: [T, 128] int64 (btile-folded).
    Returns (l1_h, l2_h, rel_h, it_h) packed idx tiles.
    """
    # destination orders
    dest_rows = np.transpose(nt, (0, 2, 1)).reshape(T, K * 128)  # [t, k*128+p]
    item_rows = items_c.reshape(T * 128)                         # [t*128+p]

    l1_blocks = np.zeros((NW, NIDX1), dtype=np.int16)
    l2_vals = np.empty((T, K * 128), dtype=np.int16)
    it_vals = np.empty(T * 128, dtype=np.int16)

    for s in range(3):
        bts = list(SEG_BTILES[s])
        content = [dest_rows[t] for t in bts]
        if s == 2:
            content.append(item_rows)
        rows_seg = np.concatenate(content)           # dest order within segment
        win = rows_seg // W
        local = (rows_seg - win * W).astype(np.int16)
        order = np.argsort(win, kind="stable")
        sorted_win = win[order]
        counts = np.bincount(sorted_win, minlength=NW)
        if counts.max() > CAPS[s]:
            raise RuntimeError(
                f"seg {s}: window count {counts.max()} exceeds cap {CAPS[s]}"
            )
        starts = np.zeros(NW, dtype=np.int64)
        starts[1:] = np.cumsum(counts)[:-1]
        pos = np.empty(len(order), dtype=np.int64)
        pos[order] = np.arange(len(order)) - starts[sorted_win]
        for w in range(NW):
            sel = order[starts[w] : starts[w] + counts[w]]
            l1_blocks[w, CAP_OFF[s] : CAP_OFF[s] + counts[w]] = local[sel]
        # staging row for each entry: window block + partition-major layout
        cslots = CAPS[s] // 128
        stg_row = win * CAPS[s] + (pos % 128) * cslots + pos // 128
        assert stg_row.max() < NW * CAPS[s] <= 32768
        stg_row = stg_row.astype(np.int16)
        off = 0
        for i, t in enumerate(bts):
            l2_vals[t] = stg_row[off : off + K * 128]
            off += K * 128
        if s == 2:
            it_vals[:] = stg_row[off : off + T * 128]

    l1_h = np.concatenate([_pack_idx16(l1_blocks[w]) for w in range(NW)], axis=1)
    l2_h = np.concatenate([_pack_idx16(l2_vals[t]) for t in range(T)], axis=1)
    rel_dest = np.transpose(nr, (0, 2, 1)).reshape(T, K * 128).astype(np.int16)
    rel_h = np.concatenate([_pack_idx16(rel_dest[t]) for t in range(T)], axis=1)
    it_h = _pack_idx16(it_vals)
    return (
        np.ascontiguousarray(l1_h),
        np.ascontiguousarray(l2_h),
        np.ascontiguousarray(rel_h),
        np.ascontiguousarray(it_h),
    )


def prepare_in_maps(
    users,
    items,
    label,
    neighbor_rel,
    neighbor_tail,
    user_emb,
    entity_emb,
    relation_emb,
    gen_W,
    gen_b,
):
    users = np.asarray(users)
    items = np.asarray(items)
    label = np.asarray(label, dtype=np.float32)
    neighbor_rel = np.asarray(neighbor_rel)
    neighbor_tail = np.asarray(neighbor_tail)

    # ---- tables (hi||lo bf16) ----
    ent2 = _hilo(entity_emb)
    pad = NW * W - N_ENTITY
    ent2 = np.concatenate([ent2, np.zeros((pad, E), BF16)], axis=0)
    ent_windows = {f"entw{w}": ent2[w * W : (w + 1) * W] for w in range(NW)}
    usr2 = _hilo(user_emb)
    rel2 = _hilo(relation_emb)

    # ---- weights: hi and lo partition halves share the same rows ----
    G = np.asarray(gen_W, dtype=np.float32).reshape(D, K, 2, D)  # [o, k, half, j]

    def packw(half):
        a = np.transpose(G[:, :, half, :], (1, 2, 0))  # [k, j, o]
        a2 = np.concatenate([a, a], axis=1)            # [k, 128, o] (hi||lo rows)
        return np.ascontiguousarray(
            np.transpose(a2, (1, 0, 2)).reshape(128, K * D).astype(BF16)
        )

    wrel_h = packw(0)
    wtail_h = packw(1)
    bias_h = np.ascontiguousarray(
        np.asarray(gen_b, np.float32).reshape(1, D).astype(BF16)
    )

    def fold(x):
        return np.ascontiguousarray(x.reshape(T, 128).T)

    in_maps = []
    for c in range(NC):
        s = slice(c * BL, (c + 1) * BL)
        nt = neighbor_tail[s].astype(np.int64).reshape(T, 128, K)
        nr = neighbor_rel[s].astype(np.int64).reshape(T, 128, K)
        items_c = items[s].astype(np.int64).reshape(T, 128)
        l1_h, l2_h, rel_h, it_h = _core_indices(nt, nr, items_c)
        in_maps.append(
            dict(
                **ent_windows,
                usr2=usr2,
                rel2=rel2,
                l1idx=l1_h,
                l2idx=l2_h,
                relidx=rel_h,
                itidx=it_h,
                uidx=fold(users[s].astype(np.int32)),
                yv=fold(label[s]),
                wtail=wtail_h,
                wrel=wrel_h,
                biasw=bias_h,
            )
        )
    return in_maps


def kernel(**inputs):
    global LAST_RESULTS
    _install_ntff_shim()
    from concourse.bass_utils import run_bass_kernel_spmd

    in_maps = prepare_in_maps(**inputs)
    nc = _get_module()
    trace = bool(int(os.environ.get("KERNEL_TRACE", "0")))
    res = run_bass_kernel_spmd(nc, in_maps, core_ids=list(range(NC)), trace=trace)
    LAST_RESULTS = res
    total = sum(float(r["loss"].sum()) for r in res.results)
    return np.float32(-total / B)
